# revision 1
# baseline (speedup 1.0000x reference)
"""BPCA pooling layer on 8 Trainium2 NeuronCores (Bass/Tile).

Math: per sample, the reference's `data = patches.reshape(-1, 4)` groups 4
consecutive channels (C=256 is divisible by 4), so `data` is exactly the
sample's contiguous buffer viewed as [N, 4] with N = H*W*C/4.  The layer is:

  1. per-column mean/std over N rows, dn = (data-mean)/std
  2. gram = dn^T dn (4x4), comp = top eigenvector (jnp.linalg.eigh)
  3. out = (dn @ comp) reshaped to [H/2, W/2, C] with channel permutation
     c' = (2*di+dj)*64 + (c//4)

Device plan (2 samples per core, pure data parallel):
  pass 1: PE computes the full 256x256 channel second-moment matrix
          M[c,c'] = sum_pix x[pix,c]*x[pix,c'] plus channel sums (ones
          column), accumulated in PSUM over all pixels, with float32r
          matmuls (2 cyc/col vs 4 for plain fp32).
  host:   fold M into the 4x4 gram (S_kl = sum_g M[4g+k,4g+l]), compute
          mean/std/gram in f64, eigh on CPU jax (same implementation the
          reference uses), derive w_k = comp_k/std_k and
          bias = -sum_k mean_k*comp_k/std_k.
  pass 2: out = sum_k x_k*w_k + bias -- a 4-op fused tensor_scalar /
          scalar_tensor_tensor chain per tile on DVE, reading host-built
          k-planes so every access and every DMA is a fully contiguous
          128-partition transfer; stores issue from the ACT queue so the
          SP queue never blocks prefetch.  The output channel permutation
          is folded into the host-side layout.
"""

import numpy as np

# ---------------------------------------------------------------------------
# Problem constants (hardcoded per spec)
# ---------------------------------------------------------------------------
B, H, W, C = 16, 112, 112, 256
N_CORES = 8
SPC = B // N_CORES          # samples per core = 2
PIX = H * W                 # 12544 pixels per sample
NBLK = PIX // 128           # 98 pixel-blocks of 128
BT = 7                      # pass-1 big tiles per sample
BLK_PER_BT = NBLK // BT     # 14 blocks per big tile
BSTRIDE = 258               # per-block SBUF cols: 256 data + 1 ones + 1 pad
NROWS = PIX * C // 4        # 802816 rows of the [N, 4] data matrix
HO, WO = H // 2, W // 2     # 56 x 56 output
T2 = 14                     # pass-2 tiles per sample (4 output rows each)
HPT = HO // T2              # 4 output rows per pass-2 tile

_programs = None
LAST_PROFILE = {}
TRACE = False
TRACE_DIRS = {}


# ---------------------------------------------------------------------------
# TileContext with a walrus-compatible tail drain
# ---------------------------------------------------------------------------
def _make_tile_context(nc):
    from concourse.tile import TileContext
    return TileContext(nc)


def _split_sync_waits(nc):
    """walrus (CoreV2/V3 codegen) rejects instructions carrying more than 2
    sync commands (waits + updates combined); Tile freely emits e.g. 2 waits
    + 1 update.  Hoist excess waits onto same-engine NOPs inserted directly
    before the offending instruction -- same engine means the same program-
    order point, so semantics are unchanged."""
    import concourse.mybir as mybir

    def mint_nop(engine):
        inner = nc.engines[engine].nop().ins
        for blk in nc.m.functions[0].blocks:
            il = blk.instructions
            for k in range(len(il) - 1, -1, -1):
                if il[k] is inner:
                    il.pop(k)
                    return inner
        raise RuntimeError("minted nop not found in any block")

    for fn in nc.m.functions:
        for blk in fn.blocks:
            il = blk.instructions
            i = 0
            while i < len(il):
                inst = il[i]
                si = inst.sync_info
                waits = list(si.on_wait) if si and si.on_wait else []
                upds = list(si.on_update) if si and si.on_update else []
                # observed walrus limits: at most 1 wait per instruction
                # (1 wait + 1 update compiles; 2 waits anywhere does not)
                if len(waits) > 1:
                    extra, keep = waits[:-1], waits[-1:]
                    for wchunk in extra:
                        nop = mint_nop(inst.engine)
                        nop.sync_info = mybir.SyncInfo(
                            on_wait=[wchunk], on_update=[])
                        il.insert(i, nop)
                        i += 1
                    inst.sync_info = mybir.SyncInfo(
                        on_wait=keep, on_update=upds)
                i += 1


def _build_pass1():
    import concourse.bass as bass
    import concourse.mybir as mybir

    f32 = mybir.dt.float32
    f32r = mybir.dt.float32r

    nc = bass.Bass("TRN2", target_bir_lowering=False, debug=False,
                   num_devices=N_CORES)
    # float32r: same bits as f32 (np dtype float32); typing the whole
    # producer chain f32r satisfies walrus's checkMatmultFP32r while the
    # PE runs the matmuls at 1 col/cycle (vs 4 for plain fp32).
    # The host pre-interleaves a ones column per block (col 256 of each
    # 258-wide block) so one DMA loads data + ones and no on-device memset
    # is needed.
    x = nc.dram_tensor("x", [SPC, 128, NBLK * BSTRIDE], f32r,
                       kind="ExternalInput").ap()
    stats = nc.dram_tensor("stats", [SPC, 2, 128, 257], f32,
                           kind="ExternalOutput").ap()

    with _make_tile_context(nc) as tc:
        with (
            tc.tile_pool(name="inp", bufs=4) as inp,
            tc.tile_pool(name="psum", bufs=2, space="PSUM") as psum,
            tc.tile_pool(name="sout", bufs=2) as soutp,
        ):
            for s in range(SPC):
                # fp32r matmuls need an even moving free size -> 258
                ps1 = psum.tile([128, BSTRIDE], f32, tag="ps1")
                ps2 = psum.tile([128, BSTRIDE], f32, tag="ps2")
                # graduated tile sizes: tiny first tile so the PE starts
                # ~4us earlier instead of waiting on a 1.75MB load
                b0 = 0
                for nb in [2, 12] + [BLK_PER_BT] * 6:
                    t = inp.tile([128, nb * BSTRIDE], f32r, tag="in")
                    t3 = t[:].rearrange("p (j b) -> p j b", b=BSTRIDE)
                    nc.sync.dma_start(
                        out=t[:],
                        in_=x[s, :, b0 * BSTRIDE:(b0 + nb) * BSTRIDE])
                    for j in range(nb):
                        first = b0 + j == 0
                        last = b0 + j == NBLK - 1
                        rhs = t3[:, j:j + 1, 0:BSTRIDE]
                        lh1 = t3[:, j:j + 1, 0:128]
                        lh2 = t3[:, j:j + 1, 128:256]
                        nc.tensor.matmul(ps1[:, 0:BSTRIDE], lh1, rhs,
                                         start=first, stop=last,
                                         skip_group_check=True)
                        nc.tensor.matmul(ps2[:, 0:BSTRIDE], lh2, rhs,
                                         start=first, stop=last,
                                         skip_group_check=True)
                    b0 += nb
                so = soutp.tile([128, 514], f32)
                nc.vector.tensor_copy(out=so[:, 0:257], in_=ps1[:, 0:257])
                nc.vector.tensor_copy(out=so[:, 257:514], in_=ps2[:, 0:257])
                # ACT-issued DMA: keeps the SP queue free to prefetch the
                # next sample's tiles (no head-of-line blocking on DVE)
                nc.scalar.dma_start(
                    out=stats[s].rearrange("t p c -> p t c"),
                    in_=so[:].rearrange("p (t c) -> p t c", c=257),
                )
    _split_sync_waits(nc)
    return nc


def _build_pass2():
    import concourse.bass as bass
    import concourse.mybir as mybir

    f32 = mybir.dt.float32
    alu = mybir.AluOpType
    OO = 49  # output pixels per partition (3136 = 64 partitions x 49)

    nc = bass.Bass("TRN2", target_bir_lowering=False, debug=False,
                   num_devices=N_CORES)
    # Partition p = (s_local*64 + p64): both samples fill 128 partitions so
    # every DMA is a fully-contiguous 128-partition transfer (~420 GB/s).
    # Host pre-transposes to per-tile k-plane blocks:
    #   x[:, off_t + (k*oo_t + oo)*C + c']
    x = nc.dram_tensor("x", [128, OO * 4 * C], f32,
                       kind="ExternalInput").ap()
    wb = nc.dram_tensor("wb", [128, 8], f32, kind="ExternalInput").ap()
    out = nc.dram_tensor("out", [128, OO * C], f32,
                         kind="ExternalOutput").ap()

    with _make_tile_context(nc) as tc:
        with (
            tc.tile_pool(name="w", bufs=1) as wpool,
            tc.tile_pool(name="inp", bufs=3) as inp,
            tc.tile_pool(name="acc", bufs=2) as accp,
        ):
            wt = wpool.tile([128, 8], f32, tag="wb")
            nc.sync.dma_start(out=wt[:], in_=wb[:])
            # touch wb on DVE so later DVE ops inherit the dep by program
            # order instead of each carrying a sem wait
            wl = wpool.tile([128, 8], f32, tag="wl")
            nc.vector.tensor_copy(out=wl[:], in_=wt[:])
            w = [wl[:, k:k + 1] for k in range(4)]
            bias = wl[:, 4:5]
            off = 0
            ooff = 0
            # tiny first tile: DVE starts ~10us earlier
            for oo in [1, 8, 8, 8, 8, 8, 8]:
                F = oo * C
                it = inp.tile([128, 4 * F], f32, tag="it")
                nc.sync.dma_start(out=it[:], in_=x[:, off:off + 4 * F])
                a0 = accp.tile([128, F], f32, tag="a0")
                a1 = accp.tile([128, F], f32, tag="a1")
                a2 = accp.tile([128, F], f32, tag="a2")
                ot = accp.tile([128, F], f32, tag="ot")
                # a0 = x0*w0 + bias; a_k = x_k*w_k + a_{k-1}; all contiguous
                nc.vector.tensor_scalar(
                    a0[:], it[:, 0:F], w[0], bias,
                    op0=alu.mult, op1=alu.add)
                nc.vector.scalar_tensor_tensor(
                    a1[:], it[:, F:2 * F], w[1], a0[:],
                    op0=alu.mult, op1=alu.add)
                nc.vector.scalar_tensor_tensor(
                    a2[:], it[:, 2 * F:3 * F], w[2], a1[:],
                    op0=alu.mult, op1=alu.add)
                nc.vector.scalar_tensor_tensor(
                    ot[:], it[:, 3 * F:4 * F], w[3], a2[:],
                    op0=alu.mult, op1=alu.add)
                # ACT-issued store: SP queue stays free to prefetch loads
                nc.scalar.dma_start(
                    out=out[:, ooff:ooff + F], in_=ot[:])
                off += 4 * F
                ooff += F
    _split_sync_waits(nc)
    return nc


def _get_programs():
    global _programs
    if _programs is None:
        _programs = (_build_pass1(), _build_pass2())
    return _programs


def _host_middle(stats):
    """stats: [B, 2, 128, 257] f32 -> w [B, 4] f64, bias [B] f64.

    Follows the reference downstream exactly: gram from (S - N mu mu^T) /
    (sigma sigma^T), comp = eigh(gram f32) top eigenvector on CPU jax.
    """
    stats = stats.astype(np.float64)
    M = np.concatenate([stats[:, 0, :, :256], stats[:, 1, :, :256]], axis=1)
    chansum = np.concatenate([stats[:, 0, :, 256], stats[:, 1, :, 256]], axis=1)

    # fold channels c = 4g+k into columns k
    Mg = M.reshape(B, 64, 4, 64, 4)
    S = np.einsum("bgkgl->bkl", Mg)                      # [B, 4, 4]
    colsum = chansum.reshape(B, 64, 4).sum(axis=1)       # [B, 4]

    mu = colsum / NROWS
    e2 = np.einsum("bkk->bk", S) / NROWS
    var = np.maximum(e2 - mu * mu, 0.0)
    sigma = np.sqrt(var)
    denom = sigma[:, :, None] * sigma[:, None, :]
    gram = (S - NROWS * mu[:, :, None] * mu[:, None, :])
    with np.errstate(divide="ignore", invalid="ignore"):
        gram = np.where(denom > 0, gram / np.where(denom > 0, denom, 1.0), 0.0)

    # eigh with the same implementation/backend the reference uses (CPU jax)
    import jax
    import jax.numpy as jnp
    with jax.default_device(jax.devices("cpu")[0]):
        V = np.asarray(jnp.linalg.eigh(jnp.asarray(gram, jnp.float32))[1])
    comp = V[:, :, -1].astype(np.float64)                # top eigenvector

    with np.errstate(divide="ignore", invalid="ignore"):
        w = np.where(sigma > 0, comp / np.where(sigma > 0, sigma, 1.0), 0.0)
    bias = -(mu * w).sum(axis=1)
    return w, bias


def kernel(x):
    from concourse.bass_utils import run_bass_kernel_spmd

    x = np.ascontiguousarray(np.asarray(x), dtype=np.float32)
    assert x.shape == (B, H, W, C), x.shape
    nc1, nc2 = _get_programs()
    core_ids = list(range(N_CORES))

    # pass-1 input: data blocks padded to 258 cols with a ones column at
    # 256, laid out exactly like the SBUF tiles ([128 partitions, blocks])
    xp = np.zeros((B, 128, NBLK, BSTRIDE), np.float32)
    xp[..., :C] = x.reshape(B, NBLK, 128, C).transpose(0, 2, 1, 3)
    xp[..., C] = 1.0
    xp = xp.reshape(B, 128, NBLK * BSTRIDE)
    in1 = [{"x": xp[c * SPC:(c + 1) * SPC]} for c in range(N_CORES)]
    kw1 = dict(trace=True, tmpdir=TRACE_DIRS.get("pass1")) if TRACE else {}
    r1 = run_bass_kernel_spmd(nc1, in1, core_ids, **kw1)
    if TRACE:
        LAST_PROFILE["pass1_ns"] = r1.exec_time_ns
    stats = np.concatenate([r1.results[c]["stats"] for c in range(N_CORES)])

    w, bias = _host_middle(stats)
    wbs = []
    for c in range(N_CORES):
        a = np.zeros((128, 8), np.float32)
        for s in range(SPC):
            b = c * SPC + s
            a[s * 64:(s + 1) * 64, 0:4] = w[b].astype(np.float32)
            a[s * 64:(s + 1) * 64, 4] = np.float32(bias[b])
        wbs.append(a)

    # pass-2 input: k-plane transpose, output-pixel-major.
    #   xplanes[s, outpix=(hi*56+wi), k, c'=(2di+dj)*64+j] = x[s,2hi+di,2wi+dj,4j+k]
    # outpix = p64*49 + oo; per tile t (oo block) the free layout is
    # [k, oo_t, c'], tiles concatenated along the free axis
    xpl = x.reshape(B, HO, 2, WO, 2, C // 4, 4).transpose(0, 1, 3, 6, 2, 4, 5)
    xpl = np.ascontiguousarray(xpl).reshape(B, 64, 49, 4, C)
    segs = []
    oo0 = 0
    for oo in [1, 8, 8, 8, 8, 8, 8]:
        seg = xpl[:, :, oo0:oo0 + oo].transpose(0, 1, 3, 2, 4)
        segs.append(seg.reshape(B, 64, 4 * oo * C))
        oo0 += oo
    x2h = np.concatenate(segs, axis=2)             # [B, 64, 49*4*C]
    in2 = []
    for c in range(N_CORES):
        pair = x2h[c * SPC:(c + 1) * SPC]          # [2, 64, 49*4*C]
        in2.append({"x": pair.reshape(128, 49 * 4 * C), "wb": wbs[c]})
    kw2 = dict(trace=True, tmpdir=TRACE_DIRS.get("pass2")) if TRACE else {}
    r2 = run_bass_kernel_spmd(nc2, in2, core_ids, **kw2)
    if TRACE:
        LAST_PROFILE["pass2_ns"] = r2.exec_time_ns

    # gather: out[s*64+p64, oo*C+c'], outpix = p64*49+oo -> [B, HO, WO, C]
    outs = [r2.results[c]["out"].reshape(SPC, HO, WO, C)
            for c in range(N_CORES)]
    return np.ascontiguousarray(np.concatenate(outs))



# revision 3
# speedup vs baseline: 1.3064x; 1.3064x over previous
"""BPCA pooling layer on 8 Trainium2 NeuronCores (Bass/Tile), bf16 pipeline.

Math: per sample, the reference's `data = patches.reshape(-1, 4)` rows are a
permutation of the sample buffer viewed as [N, 4] (N = H*W*C/4); mean/std/gram
are row-order invariant, so any enumeration of (pixel, channel-group) rows
works for the stats.  The layer is:

  1. per-column mean/std over N rows, dn = (data-mean)/std
  2. gram = dn^T dn (4x4), comp = top eigenvector (jnp.linalg.eigh)
  3. out = (dn @ comp) reshaped to [H/2, W/2, C] with channel permutation

Device plan (2 samples per core, pure data parallel, ONE shared bf16 input
array for both passes -- rel err ~3.3e-3 vs the 2e-2 gate, validated by
simulation):

  host prep: D-layout, k-major groups.  D[r,k] = x.flat[4r+k]; rows are
          blocked r = j*128 + p (p = partition).  Groups of 32 j-blocks give
          tiles [128, 130]: cols k*32+jl = D[(g*32+jl)*128+p, k], col 128 =
          ones, col 129 = pad.  bf16, fully contiguous DMA.
  pass 1: per group ONE bf16 matmul psum += lhsT(t[:,0:128])^T @ rhs(t[:,0:130])
          accumulating the [128,130] block-product matrix in PSUM over 196
          groups; its (k*32+jl, l*32+jl) diagonal entries fold to the 4x4
          second-moment matrix S, col 128 gives channel sums.  PE work =
          2*196*130 cycles ~ 21us @ 2.4GHz, under the ~39us DMA.
  host:   fold diag, f64 stats, CPU-jax eigh (same implementation as the
          reference -> same eigenvector sign), w_k = comp_k/std_k,
          bias = -sum mean_k w_k.
  pass 2: out = sum_k x_k*w_k + bias over the SAME bf16 array; per tile the
          4-term MAC chain runs DVE -> Pool -> Pool -> DVE (tensor_scalar /
          scalar_tensor_tensor, per-partition scalar APs), bf16 accumulators,
          bf16 output (host upcasts + unscrambles the layout for free).
"""

import numpy as np

# ---------------------------------------------------------------------------
# Problem constants (hardcoded per spec)
# ---------------------------------------------------------------------------
B, H, W, C = 16, 112, 112, 256
N_CORES = 8
SPC = B // N_CORES          # samples per core = 2
NROWS = H * W * C // 4      # 802816 rows of the [N, 4] data matrix
NBLK = NROWS // 128         # 6272 row-blocks of 128
GRP = 32                    # j-blocks per matmul group
NG = NBLK // GRP            # 196 groups per sample
GC = 4 * GRP + 2            # 130 cols per group: 128 data + ones + pad
HO, WO = H // 2, W // 2     # 56 x 56 output

P1_CHUNKS = [4, 24] + [28] * 6          # sums to 196; tiny first tile
P2_CHUNKS = [2, 12] + [14] * 13         # sums to 196

_programs = None
LAST_PROFILE = {}
TRACE = False
TRACE_DIRS = {}


def _bf16():
    import ml_dtypes
    return ml_dtypes.bfloat16


# ---------------------------------------------------------------------------
# TileContext helpers
# ---------------------------------------------------------------------------
def _make_tile_context(nc):
    from concourse.tile import TileContext
    return TileContext(nc)


def _split_sync_waits(nc):
    """walrus (CoreV2/V3 codegen) rejects instructions carrying more than 2
    sync commands (waits + updates combined); Tile freely emits e.g. 2 waits
    + 1 update.  Hoist excess waits onto same-engine NOPs inserted directly
    before the offending instruction -- same engine means the same program-
    order point, so semantics are unchanged."""
    import concourse.mybir as mybir

    def mint_nop(engine):
        inner = nc.engines[engine].nop().ins
        for blk in nc.m.functions[0].blocks:
            il = blk.instructions
            for k in range(len(il) - 1, -1, -1):
                if il[k] is inner:
                    il.pop(k)
                    return inner
        raise RuntimeError("minted nop not found in any block")

    for fn in nc.m.functions:
        for blk in fn.blocks:
            il = blk.instructions
            i = 0
            while i < len(il):
                inst = il[i]
                si = inst.sync_info
                waits = list(si.on_wait) if si and si.on_wait else []
                upds = list(si.on_update) if si and si.on_update else []
                # observed walrus limits: at most 1 wait per instruction
                if len(waits) > 1:
                    extra, keep = waits[:-1], waits[-1:]
                    for wchunk in extra:
                        nop = mint_nop(inst.engine)
                        nop.sync_info = mybir.SyncInfo(
                            on_wait=[wchunk], on_update=[])
                        il.insert(i, nop)
                        i += 1
                    inst.sync_info = mybir.SyncInfo(
                        on_wait=keep, on_update=upds)
                i += 1


def _build_pass1():
    import concourse.bass as bass
    import concourse.mybir as mybir

    f32 = mybir.dt.float32
    bf16 = mybir.dt.bfloat16

    nc = bass.Bass("TRN2", target_bir_lowering=False, debug=False,
                   num_devices=N_CORES)
    x = nc.dram_tensor("x", [SPC, 128, NG * GC], bf16,
                       kind="ExternalInput").ap()
    stats = nc.dram_tensor("stats", [SPC, 128, GC], f32,
                           kind="ExternalOutput").ap()

    with _make_tile_context(nc) as tc:
        with (
            tc.tile_pool(name="inp", bufs=4) as inp,
            tc.tile_pool(name="psum", bufs=2, space="PSUM") as psum,
            tc.tile_pool(name="sout", bufs=2) as soutp,
        ):
            for s in range(SPC):
                ps = psum.tile([128, GC], f32, tag=f"ps{s}")
                g0 = 0
                for ng in P1_CHUNKS:
                    t = inp.tile([128, ng * GC], bf16, tag="in")
                    t3 = t[:].rearrange("p (g c) -> p g c", c=GC)
                    nc.sync.dma_start(
                        out=t[:], in_=x[s, :, g0 * GC:(g0 + ng) * GC])
                    for j in range(ng):
                        nc.tensor.matmul(
                            ps[:, 0:GC],
                            t3[:, j, 0:128],
                            t3[:, j, 0:GC],
                            start=(g0 + j == 0),
                            stop=(g0 + j == NG - 1),
                            skip_group_check=True)
                    g0 += ng
                so = soutp.tile([128, GC], f32, tag="so")
                nc.vector.tensor_copy(out=so[:], in_=ps[:, 0:GC])
                # ACT-issued DMA keeps the SP queue free for prefetch
                nc.scalar.dma_start(out=stats[s], in_=so[:])
    _split_sync_waits(nc)
    return nc


def _build_pass2():
    import concourse.bass as bass
    import concourse.mybir as mybir

    f32 = mybir.dt.float32
    bf16 = mybir.dt.bfloat16
    alu = mybir.AluOpType
    AF = mybir.ActivationFunctionType

    nc = bass.Bass("TRN2", target_bir_lowering=False, debug=False,
                   num_devices=N_CORES)
    x = nc.dram_tensor("x", [SPC, 128, NG * GC], bf16,
                       kind="ExternalInput").ap()
    wb = nc.dram_tensor("wb", [128, 16], f32, kind="ExternalInput").ap()
    out = nc.dram_tensor("out", [SPC, 128, NBLK], bf16,
                         kind="ExternalOutput").ap()

    with _make_tile_context(nc) as tc:
        with (
            tc.tile_pool(name="w", bufs=1) as wpool,
            tc.tile_pool(name="inp", bufs=3) as inp,
            tc.tile_pool(name="acc", bufs=2) as accp,
        ):
            wt = wpool.tile([128, 16], f32, tag="wb")
            nc.sync.dma_start(out=wt[:], in_=wb[:])
            for s in range(SPC):
                w = [wt[:, 8 * s + k:8 * s + k + 1] for k in range(4)]
                bias = wt[:, 8 * s + 4:8 * s + 5]
                g0 = 0
                for ng in P2_CHUNKS:
                    F = ng * GRP
                    t = inp.tile([128, ng * GC], bf16, tag="it")
                    t3 = t[:].rearrange("p (g c) -> p g c", c=GC)
                    nc.sync.dma_start(
                        out=t[:], in_=x[s, :, g0 * GC:(g0 + ng) * GC])
                    a0 = accp.tile([128, F], bf16, tag="a0")
                    a1 = accp.tile([128, F], bf16, tag="a1")
                    a2 = accp.tile([128, F], bf16, tag="a2")
                    a3 = accp.tile([128, F], bf16, tag="a3")
                    o = accp.tile([128, F], bf16, tag="o")
                    v = [a[:].rearrange("p (g c) -> p g c", c=GRP)
                         for a in (a0, a1, a2, a3, o)]
                    xk = [t3[:, :, GRP * k:GRP * (k + 1)] for k in range(4)]
                    # two parallel branches, final add on Pool:
                    #   a0 = x0*w0 + bias (ACT), a1 = x1*w1 (ACT)
                    #   a2 = x2*w2 + a0 (DVE),   a3 = x3*w3 + a1 (DVE)
                    #   o = a2 + a3 (Pool)
                    nc.scalar.activation(
                        v[0], xk[0], AF.Identity, bias=bias, scale=w[0])
                    nc.scalar.activation(
                        v[1], xk[1], AF.Copy, bias=0.0, scale=w[1])
                    nc.vector.scalar_tensor_tensor(
                        v[2], xk[2], w[2], v[0], op0=alu.mult, op1=alu.add)
                    nc.vector.scalar_tensor_tensor(
                        v[3], xk[3], w[3], v[1], op0=alu.mult, op1=alu.add)
                    nc.gpsimd.tensor_tensor(
                        out=v[4], in0=v[2], in1=v[3], op=alu.add)
                    # Pool-issued store: SP prefetches, ACT/DVE compute
                    nc.gpsimd.dma_start(
                        out=out[s, :, g0 * GRP:g0 * GRP + F], in_=o[:])
                    g0 += ng
    _split_sync_waits(nc)
    return nc


def _get_programs():
    global _programs
    if _programs is None:
        _programs = (_build_pass1(), _build_pass2())
    return _programs


def _host_prep(x):
    """x [B,H,W,C] f32 -> bf16 D-layout groups [B, 128, NG*GC]."""
    bf16 = _bf16()
    xg = np.empty((B, 128, NG, GC), bf16)
    d = x.reshape(B, NBLK, 128, 4).transpose(0, 2, 1, 3)      # [B,128,j,k]
    d = d.reshape(B, 128, NG, GRP, 4).transpose(0, 1, 2, 4, 3)  # [B,128,g,k,jl]
    xg[..., :128] = d.reshape(B, 128, NG, 128).astype(bf16)
    xg[..., 128] = 1.0
    xg[..., 129] = 0.0
    return xg.reshape(B, 128, NG * GC)


def _host_middle(stats):
    """stats: [B, 128, GC] f32 -> w [B, 4] f64, bias [B] f64.

    PSUM[(k*32+jl), (l*32+jl')] = block products; diagonal jl==jl' entries
    fold to S_kl, col 128 folds to channel sums.  Downstream matches the
    reference exactly: gram from (S - N mu mu^T)/(sigma sigma^T), comp =
    eigh(gram f32) top eigenvector on CPU jax.
    """
    st = stats.astype(np.float64)
    S = np.einsum("bkjlj->bkl", st[:, :, :128].reshape(B, 4, GRP, 4, GRP))
    colsum = st[:, :, 128].reshape(B, 4, GRP).sum(axis=2)

    mu = colsum / NROWS
    e2 = np.einsum("bkk->bk", S) / NROWS
    var = np.maximum(e2 - mu * mu, 0.0)
    sigma = np.sqrt(var)
    denom = sigma[:, :, None] * sigma[:, None, :]
    gram = (S - NROWS * mu[:, :, None] * mu[:, None, :])
    with np.errstate(divide="ignore", invalid="ignore"):
        gram = np.where(denom > 0, gram / np.where(denom > 0, denom, 1.0), 0.0)

    # eigh with the same implementation/backend the reference uses (CPU jax)
    import jax
    import jax.numpy as jnp
    with jax.default_device(jax.devices("cpu")[0]):
        V = np.asarray(jnp.linalg.eigh(jnp.asarray(gram, jnp.float32))[1])
    comp = V[:, :, -1].astype(np.float64)                # top eigenvector

    with np.errstate(divide="ignore", invalid="ignore"):
        w = np.where(sigma > 0, comp / np.where(sigma > 0, sigma, 1.0), 0.0)
    bias = -(mu * w).sum(axis=1)
    return w, bias


def kernel(x):
    from concourse.bass_utils import run_bass_kernel_spmd

    x = np.ascontiguousarray(np.asarray(x), dtype=np.float32)
    assert x.shape == (B, H, W, C), x.shape
    nc1, nc2 = _get_programs()
    core_ids = list(range(N_CORES))

    xg = _host_prep(x)
    in1 = [{"x": xg[c * SPC:(c + 1) * SPC]} for c in range(N_CORES)]
    kw1 = dict(trace=True, tmpdir=TRACE_DIRS.get("pass1")) if TRACE else {}
    r1 = run_bass_kernel_spmd(nc1, in1, core_ids, **kw1)
    if TRACE:
        LAST_PROFILE["pass1_ns"] = r1.exec_time_ns
    stats = np.concatenate([r1.results[c]["stats"] for c in range(N_CORES)])

    w, bias = _host_middle(stats)
    wbs = []
    for c in range(N_CORES):
        a = np.zeros((128, 16), np.float32)
        for s in range(SPC):
            b = c * SPC + s
            a[:, 8 * s:8 * s + 4] = w[b].astype(np.float32)
            a[:, 8 * s + 4] = np.float32(bias[b])
        wbs.append(a)

    in2 = [{"x": xg[c * SPC:(c + 1) * SPC], "wb": wbs[c]}
           for c in range(N_CORES)]
    kw2 = dict(trace=True, tmpdir=TRACE_DIRS.get("pass2")) if TRACE else {}
    r2 = run_bass_kernel_spmd(nc2, in2, core_ids, **kw2)
    if TRACE:
        LAST_PROFILE["pass2_ns"] = r2.exec_time_ns

    # gather + unscramble: dev out [B, p=(dj,cg), j=(h,wo)] -> [B,HO,WO,C]
    dev = np.concatenate([np.asarray(r2.results[c]["out"])
                          for c in range(N_CORES)])
    dev = dev.astype(np.float32).reshape(B, 2, 64, HO, 2, WO)
    #                 [b, dj, cg, ho, di, wo] -> [b, ho, wo, di, dj, cg]
    out = dev.transpose(0, 3, 5, 4, 1, 2).reshape(B, HO, WO, C)
    return np.ascontiguousarray(out)


# revision 8
# speedup vs baseline: 1.5054x; 1.1524x over previous
"""BPCA pooling layer on 8 Trainium2 NeuronCores (Bass/Tile), bf16 pipeline.

Math: per sample, the reference's `data = patches.reshape(-1, 4)` rows are a
permutation of the sample buffer viewed as [N, 4] (N = H*W*C/4); mean/std/gram
are row-order invariant, so any enumeration of (pixel, channel-group) rows
works for the stats.  The layer is:

  1. per-column mean/std over N rows, dn = (data-mean)/std
  2. gram = dn^T dn (4x4), comp = top eigenvector (jnp.linalg.eigh)
  3. out = (dn @ comp) reshaped to [H/2, W/2, C] with channel permutation

Device plan (2 samples per core, pure data parallel, ONE shared bf16 input
array for both passes -- rel err ~3.3e-3 vs the 2e-2 gate, validated by
simulation):

  host prep: D-layout, k-major groups.  D[r,k] = x.flat[4r+k]; rows are
          blocked r = j*128 + p (p = partition).  Groups of 32 j-blocks give
          tiles [128, 130]: cols k*32+jl = D[(g*32+jl)*128+p, k], col 128 =
          ones, col 129 = pad.  bf16, fully contiguous DMA.
  pass 1: per group ONE bf16 matmul psum += lhsT(t[:,0:128])^T @ rhs(t[:,0:130])
          accumulating the [128,130] block-product matrix in PSUM over 196
          groups; its (k*32+jl, l*32+jl) diagonal entries fold to the 4x4
          second-moment matrix S, col 128 gives channel sums.  PE work =
          2*196*130 cycles ~ 21us @ 2.4GHz, under the ~39us DMA.
  host:   fold diag, f64 stats, CPU-jax eigh (same implementation as the
          reference -> same eigenvector sign), w_k = comp_k/std_k,
          bias = -sum mean_k w_k.
  pass 2: out = sum_k x_k*w_k + bias over the SAME bf16 array; per tile the
          4-term MAC chain runs DVE -> Pool -> Pool -> DVE (tensor_scalar /
          scalar_tensor_tensor, per-partition scalar APs), bf16 accumulators,
          bf16 output (host upcasts + unscrambles the layout for free).
"""

import numpy as np

# ---------------------------------------------------------------------------
# Problem constants (hardcoded per spec)
# ---------------------------------------------------------------------------
B, H, W, C = 16, 112, 112, 256
N_CORES = 8
SPC = B // N_CORES          # samples per core = 2
NROWS = H * W * C // 4      # 802816 rows of the [N, 4] data matrix
NBLK = NROWS // 128         # 6272 row-blocks of 128
GRP = 32                    # j-blocks per matmul group
NG = NBLK // GRP            # 196 groups per sample
GC = 4 * GRP + 2            # 130 cols per group: 128 data + ones + pad
HO, WO = H // 2, W // 2     # 56 x 56 output

P1_CHUNKS = [2, 12, 26] + [28] * 4 + [22, 22]   # sums to 196; tiny first tile
P2_CHUNKS = [2, 12] + [26] * 7                  # sums to 196 (groups of 32 blks)

_programs = None
LAST_PROFILE = {}
TRACE = False
TRACE_DIRS = {}


def _bf16():
    import ml_dtypes
    return ml_dtypes.bfloat16


# ---------------------------------------------------------------------------
# TileContext helpers
# ---------------------------------------------------------------------------
def _make_tile_context(nc):
    from concourse.tile import TileContext
    return TileContext(nc)


def _split_sync_waits(nc):
    """walrus (CoreV2/V3 codegen) rejects instructions carrying more than 2
    sync commands (waits + updates combined); Tile freely emits e.g. 2 waits
    + 1 update.  Hoist excess waits onto same-engine NOPs inserted directly
    before the offending instruction -- same engine means the same program-
    order point, so semantics are unchanged."""
    import concourse.mybir as mybir

    def mint_nop(engine):
        inner = nc.engines[engine].nop().ins
        for blk in nc.m.functions[0].blocks:
            il = blk.instructions
            for k in range(len(il) - 1, -1, -1):
                if il[k] is inner:
                    il.pop(k)
                    return inner
        raise RuntimeError("minted nop not found in any block")

    for fn in nc.m.functions:
        for blk in fn.blocks:
            il = blk.instructions
            i = 0
            while i < len(il):
                inst = il[i]
                si = inst.sync_info
                waits = list(si.on_wait) if si and si.on_wait else []
                upds = list(si.on_update) if si and si.on_update else []
                # observed walrus limits: at most 1 wait per instruction
                if len(waits) > 1:
                    extra, keep = waits[:-1], waits[-1:]
                    for wchunk in extra:
                        nop = mint_nop(inst.engine)
                        nop.sync_info = mybir.SyncInfo(
                            on_wait=[wchunk], on_update=[])
                        il.insert(i, nop)
                        i += 1
                    inst.sync_info = mybir.SyncInfo(
                        on_wait=keep, on_update=upds)
                i += 1


def _build_pass1():
    import concourse.bass as bass
    import concourse.mybir as mybir

    f32 = mybir.dt.float32
    bf16 = mybir.dt.bfloat16

    nc = bass.Bass("TRN2", target_bir_lowering=False, debug=False,
                   num_devices=N_CORES)
    x = nc.dram_tensor("x", [SPC, 128, NG * GC], bf16,
                       kind="ExternalInput").ap()
    stats = nc.dram_tensor("stats", [SPC, 128, GC], f32,
                           kind="ExternalOutput").ap()

    with _make_tile_context(nc) as tc:
        with (
            tc.tile_pool(name="inp", bufs=4) as inp,
            tc.tile_pool(name="psum", bufs=2, space="PSUM") as psum,
            tc.tile_pool(name="sout", bufs=2) as soutp,
        ):
            for s in range(SPC):
                ps = psum.tile([128, GC], f32, tag=f"ps{s}")
                g0 = 0
                for ng in P1_CHUNKS:
                    t = inp.tile([128, ng * GC], bf16, tag="in")
                    t3 = t[:].rearrange("p (g c) -> p g c", c=GC)
                    nc.sync.dma_start(
                        out=t[:], in_=x[s, :, g0 * GC:(g0 + ng) * GC])
                    for j in range(ng):
                        nc.tensor.matmul(
                            ps[:, 0:GC],
                            t3[:, j, 0:128],
                            t3[:, j, 0:GC],
                            start=(g0 + j == 0),
                            stop=(g0 + j == NG - 1),
                            skip_group_check=True)
                    g0 += ng
                so = soutp.tile([128, GC], f32, tag="so")
                nc.vector.tensor_copy(out=so[:], in_=ps[:, 0:GC])
                # ACT-issued DMA keeps the SP queue free for prefetch
                nc.scalar.dma_start(out=stats[s], in_=so[:])
    _split_sync_waits(nc)
    return nc


def _build_pass2():
    import concourse.bass as bass
    import concourse.mybir as mybir

    f32 = mybir.dt.float32
    bf16 = mybir.dt.bfloat16
    alu = mybir.AluOpType
    AF = mybir.ActivationFunctionType

    nc = bass.Bass("TRN2", target_bir_lowering=False, debug=False,
                   num_devices=N_CORES)
    # tile-segmented k-plane layout: per tile [128, 4*NB] with plane k at
    # cols [k*NB, (k+1)*NB) -- every compute op reads/writes dense runs
    x = nc.dram_tensor("x", [SPC, 128, 4 * NBLK], bf16,
                       kind="ExternalInput").ap()
    wb = nc.dram_tensor("wb", [128, 16], f32, kind="ExternalInput").ap()
    out = nc.dram_tensor("out", [SPC, 128, NBLK], bf16,
                         kind="ExternalOutput").ap()

    with _make_tile_context(nc) as tc:
        with (
            tc.tile_pool(name="w", bufs=1) as wpool,
            tc.tile_pool(name="inp", bufs=3) as inp,
            tc.tile_pool(name="acc", bufs=2) as accp,
        ):
            wt = wpool.tile([128, 16], f32, tag="wb")
            nc.sync.dma_start(out=wt[:], in_=wb[:])
            for s in range(SPC):
                w = [wt[:, 8 * s + k:8 * s + k + 1] for k in range(4)]
                bias = wt[:, 8 * s + 4:8 * s + 5]
                b0 = 0
                for ng in P2_CHUNKS:
                    NB = ng * GRP
                    t = inp.tile([128, 4 * NB], bf16, tag="it")
                    nc.sync.dma_start(
                        out=t[:], in_=x[s, :, 4 * b0:4 * (b0 + NB)])
                    a0 = accp.tile([128, NB], bf16, tag="a0")
                    a1 = accp.tile([128, NB], bf16, tag="a1")
                    a2 = accp.tile([128, NB], bf16, tag="a2")
                    a3 = accp.tile([128, NB], bf16, tag="a3")
                    o = accp.tile([128, NB], bf16, tag="o")
                    xk = [t[:, NB * k:NB * (k + 1)] for k in range(4)]
                    # two parallel branches, final add on Pool:
                    #   a0 = x0*w0 + bias (ACT), a1 = x1*w1 (ACT)
                    #   a2 = x2*w2 + a0 (DVE),   a3 = x3*w3 + a1 (DVE)
                    #   o = a2 + a3 (Pool)
                    nc.scalar.activation(
                        a0[:], xk[0], AF.Identity, bias=bias, scale=w[0])
                    nc.scalar.activation(
                        a1[:], xk[1], AF.Copy, bias=0.0, scale=w[1])
                    nc.vector.scalar_tensor_tensor(
                        a2[:], xk[2], w[2], a0[:], op0=alu.mult, op1=alu.add)
                    nc.vector.scalar_tensor_tensor(
                        a3[:], xk[3], w[3], a1[:], op0=alu.mult, op1=alu.add)
                    nc.gpsimd.tensor_tensor(
                        out=o[:], in0=a2[:], in1=a3[:], op=alu.add)
                    # ACT-issued store: SP stays free for prefetch
                    nc.scalar.dma_start(
                        out=out[s, :, b0:b0 + NB], in_=o[:])
                    b0 += NB
    _split_sync_waits(nc)
    return nc


def _get_programs():
    global _programs
    if _programs is None:
        _programs = (_build_pass1(), _build_pass2())
    return _programs


def _host_prep(x):
    """x [B,H,W,C] f32 -> bf16 D-layout groups [B, 128, NG*GC]."""
    bf16 = _bf16()
    xg = np.empty((B, 128, NG, GC), bf16)
    d = x.reshape(B, NBLK, 128, 4).transpose(0, 2, 1, 3)      # [B,128,j,k]
    d = d.reshape(B, 128, NG, GRP, 4).transpose(0, 1, 2, 4, 3)  # [B,128,g,k,jl]
    xg[..., :128] = d.reshape(B, 128, NG, 128).astype(bf16)
    xg[..., 128] = 1.0
    xg[..., 129] = 0.0
    return xg.reshape(B, 128, NG * GC)


def _host_prep2(x):
    """x [B,H,W,C] f32 -> bf16 tile-segmented k-plane layout [B,128,4*NBLK].

    Per tile (chunk of NB = ng*32 blocks): [128, 4, NB] with plane k dense.
    """
    bf16 = _bf16()
    xp = x.reshape(B, NBLK, 128, 4).transpose(0, 2, 3, 1).astype(bf16)
    xt = np.empty((B, 128, 4 * NBLK), bf16)                # [B,128,k,b]
    b0 = 0
    for ng in P2_CHUNKS:
        NB = ng * GRP
        xt[:, :, 4 * b0:4 * (b0 + NB)] = \
            xp[:, :, :, b0:b0 + NB].reshape(B, 128, 4 * NB)
        b0 += NB
    return xt


def _host_middle(stats):
    """stats: [B, 128, GC] f32 -> w [B, 4] f64, bias [B] f64.

    PSUM[(k*32+jl), (l*32+jl')] = block products; diagonal jl==jl' entries
    fold to S_kl, col 128 folds to channel sums.  Downstream matches the
    reference exactly: gram from (S - N mu mu^T)/(sigma sigma^T), comp =
    eigh(gram f32) top eigenvector on CPU jax.
    """
    st = stats.astype(np.float64)
    S = np.einsum("bkjlj->bkl", st[:, :, :128].reshape(B, 4, GRP, 4, GRP))
    colsum = st[:, :, 128].reshape(B, 4, GRP).sum(axis=2)

    mu = colsum / NROWS
    e2 = np.einsum("bkk->bk", S) / NROWS
    var = np.maximum(e2 - mu * mu, 0.0)
    sigma = np.sqrt(var)
    denom = sigma[:, :, None] * sigma[:, None, :]
    gram = (S - NROWS * mu[:, :, None] * mu[:, None, :])
    with np.errstate(divide="ignore", invalid="ignore"):
        gram = np.where(denom > 0, gram / np.where(denom > 0, denom, 1.0), 0.0)

    # eigh with the same implementation/backend the reference uses (CPU jax)
    import jax
    import jax.numpy as jnp
    with jax.default_device(jax.devices("cpu")[0]):
        V = np.asarray(jnp.linalg.eigh(jnp.asarray(gram, jnp.float32))[1])
    comp = V[:, :, -1].astype(np.float64)                # top eigenvector

    with np.errstate(divide="ignore", invalid="ignore"):
        w = np.where(sigma > 0, comp / np.where(sigma > 0, sigma, 1.0), 0.0)
    bias = -(mu * w).sum(axis=1)
    return w, bias


def kernel(x):
    from concourse.bass_utils import run_bass_kernel_spmd

    x = np.ascontiguousarray(np.asarray(x), dtype=np.float32)
    assert x.shape == (B, H, W, C), x.shape
    nc1, nc2 = _get_programs()
    core_ids = list(range(N_CORES))

    xg = _host_prep(x)
    in1 = [{"x": xg[c * SPC:(c + 1) * SPC]} for c in range(N_CORES)]
    kw1 = dict(trace=True, tmpdir=TRACE_DIRS.get("pass1")) if TRACE else {}
    r1 = run_bass_kernel_spmd(nc1, in1, core_ids, **kw1)
    if TRACE:
        LAST_PROFILE["pass1_ns"] = r1.exec_time_ns
    stats = np.concatenate([r1.results[c]["stats"] for c in range(N_CORES)])

    w, bias = _host_middle(stats)
    wbs = []
    for c in range(N_CORES):
        a = np.zeros((128, 16), np.float32)
        for s in range(SPC):
            b = c * SPC + s
            a[:, 8 * s:8 * s + 4] = w[b].astype(np.float32)
            a[:, 8 * s + 4] = np.float32(bias[b])
        wbs.append(a)

    xt2 = _host_prep2(x)
    in2 = [{"x": xt2[c * SPC:(c + 1) * SPC], "wb": wbs[c]}
           for c in range(N_CORES)]
    kw2 = dict(trace=True, tmpdir=TRACE_DIRS.get("pass2")) if TRACE else {}
    r2 = run_bass_kernel_spmd(nc2, in2, core_ids, **kw2)
    if TRACE:
        LAST_PROFILE["pass2_ns"] = r2.exec_time_ns

    # gather + unscramble: dev out [B, p=(dj,cg), j=(h,wo)] -> [B,HO,WO,C]
    dev = np.concatenate([np.asarray(r2.results[c]["out"])
                          for c in range(N_CORES)])
    dev = dev.astype(np.float32).reshape(B, 2, 64, HO, 2, WO)
    #                 [b, dj, cg, ho, di, wo] -> [b, ho, wo, di, dj, cg]
    out = dev.transpose(0, 3, 5, 4, 1, 2).reshape(B, HO, WO, C)
    return np.ascontiguousarray(out)
